# revision 44
# baseline (speedup 1.0000x reference)
"""Trainium2 Bass kernel for nn_AttentionBlock (B=16, C=512, H=W=32, 8 heads).

Data-parallel over batch: 16 batches / 8 cores = 2 per core.

v2 design (vs baseline):
  - x converted to bf16 on host: halves input DMA, removes bf16-staging
    copies on ScalarE, enables 2x DVE modes for LN elementwise ops.
  - S matmuls (K=64 per head) row-tiled: the two heads of a pair run on
    PE tiles (0,0)/(64,0) concurrently -> ~2x on the S phase.
  - One exp per (pair, st, half) over [128, 1024] PSUM (covers both heads).
  - Softmax denominator via the AV ones-columns trick, then ONE
    reciprocal per (head) [1, 1024], DRAM-bounce broadcast to 64
    partitions, single multiply per (head, half) for the h eviction.
    (Replaces baseline's per-(head,half) recip/copy/recip/mul chain.)
  - Both batches' LN stats (the only non-exp ScalarE table users) run
    before the first exp: zero activation-table swaps in steady state.
  - Stationary reuse: LN stats share one ones ldweights; QKV/proj
    accumulate cc-outer/half-inner so each weight chunk loads once.
  - v2 ones tiles persist across calls (memset once at start).

All matmuls bf16 (fp32 PSUM accumulation). I/O: x bf16 (host-cast),
out fp32.
"""

import math

import numpy as np
import ml_dtypes

import concourse.bass as bass
import concourse.bacc as bacc
import concourse.tile as tile
from concourse import mybir
from concourse.bass_utils import run_bass_kernel_spmd

P = 128
C = 512
T = 1024
N_HEADS = 8
HD = 64
B = 16
N_CORES = 8
B_LOC = B // N_CORES  # batches per core
CCH = C // P  # channel chunks of 128
EPS = 1e-5

F32 = mybir.dt.float32
BF16 = mybir.dt.bfloat16
FP8 = mybir.dt.float8e4
LN16 = math.log(16.0)

HALVES = ((0, slice(0, 512)), (1, slice(512, 1024)))


def _interleave(*seqs):
    """Proportional merge of chunk lists (stable within each list)."""
    items = []
    for si, s in enumerate(seqs):
        n = max(len(s), 1)
        for i, c in enumerate(s):
            items.append(((i + 0.5) / n, si, c))
    items.sort(key=lambda t: (t[0], t[1]))
    return [c for _, _, c in items]


def _emit(tc, nc, pools, aps, dbg=None):
    mul = mybir.AluOpType.mult
    add = mybir.AluOpType.add
    sub = mybir.AluOpType.subtract

    x_d, wqk_d, wv_d, wp_d, bqk_d, bv_d, bp_d, out_d = aps
    (const, xpool, x2pool, xnpool, statp, qkpool, hpool, expp, rdsp, rdbp, outp,
     psp, accp, drp) = pools

    # DRAM views
    xv = x_d.rearrange("b (cc p) t -> b p cc t", p=P)
    ov = out_d.rearrange("b (cc p) t -> b p cc t", p=P)

    # ---- persistent tiles ----
    wqk_sb = const.tile([P, CCH, 2 * C], BF16)
    wv_sb = const.tile([P, CCH, C], BF16)
    wp_sb = const.tile([P, CCH, C], BF16)
    bqk_sb = const.tile([P, 2 * C // P], F32)
    bp_sb = const.tile([P, CCH], F32)
    bv_b = const.tile([P, C], F32)
    ones_b = const.tile([P, P], BF16)
    eps_sb = const.tile([P, 1], F32)
    nln16_sb = const.tile([P, 1], F32)
    # per-batch v2 tiles: [t-chunk partitions, st, head*128 + (data|ones)]
    # even head: v data in cols 0:64 (ones in 64:128); odd head reversed.
    v2_t = [
        const.tile([P, 8, N_HEADS * P], BF16, name=f"v2_{b}") for b in range(B_LOC)
    ]

    def emit_consts():
        nc.vector.memset(ones_b, 1.0)
        nc.vector.memset(eps_sb, EPS)
        nc.vector.memset(nln16_sb, -LN16)
        for b in range(B_LOC):
            # gpsimd: slow but fully parallel to the DVE-heavy startup
            nc.gpsimd.memset(v2_t[b], 1.0)
        nc.sync.dma_start(wqk_sb, wqk_d.rearrange("(cc p) o -> p cc o", p=P))
        nc.sync.dma_start(wv_sb, wv_d.rearrange("(cc p) o -> p cc o", p=P))
        nc.sync.dma_start(bqk_sb, bqk_d.rearrange("(o p) -> p o", p=P))
        nc.sync.dma_start(
            bv_b,
            bass.AP(tensor=bv_d.tensor, offset=bv_d.offset, ap=[[0, P]] + list(bv_d.ap)),
        )
        nc.sync.dma_start(bp_sb, bp_d.rearrange("(o p) -> p o", p=P))
        nc.sync.dma_start(wp_sb, wp_d.rearrange("(cc p) o -> p cc o", p=P))

    state = [dict() for _ in range(B_LOC)]

    # ---------------- phase A: LN + QKV ----------------
    def chunks_load(b):
        S = state[b]

        def c_load():
            S["x"] = xpool.tile([P, CCH, T], BF16, tag="x", name="x_t")
            for cc in range(CCH):
                nc.sync.dma_start(S["x"][:, cc], xv[b, :, cc])

        return [c_load]

    def chunks_stats(b):
        S = state[b]
        ch = []

        def c_sq(cc):
            if "x2" not in S:
                S["x2"] = x2pool.tile([P, CCH, T], BF16, tag="x2", name="x2_t")
            nc.vector.tensor_tensor(S["x2"][:, cc], S["x"][:, cc], S["x"][:, cc], mul)

        for cc in range(CCH):
            ch.append(lambda cc=cc: c_sq(cc))

        def c_statmm():
            S["muB"] = psp.tile([P, T], F32, tag="ps", name="ps_t")
            S["sqB"] = psp.tile([P, T], F32, tag="ps", name="ps_t")
            # all 16 matmuls share the ones stationary
            for _, hs in HALVES:
                for cc in range(CCH):
                    nc.tensor.matmul(
                        S["muB"][:, hs], ones_b, S["x"][:, cc, hs],
                        start=(cc == 0), stop=(cc == CCH - 1),
                    )
            for _, hs in HALVES:
                for cc in range(CCH):
                    nc.tensor.matmul(
                        S["sqB"][:, hs], ones_b, S["x2"][:, cc, hs],
                        start=(cc == 0), stop=(cc == CCH - 1),
                    )

        ch.append(c_statmm)

        def c_statev():
            m_bf = statp.tile([P, T], BF16, tag="stat", name="stat_t")
            nc.vector.tensor_scalar_mul(m_bf, S["muB"], 1.0 / C)
            m2 = statp.tile([P, T], BF16, tag="stat", name="stat_t")
            nc.vector.tensor_tensor(m2, m_bf, m_bf, mul)
            var = statp.tile([P, T], F32, tag="stat", name="stat_t")
            nc.vector.scalar_tensor_tensor(var, S["sqB"], 1.0 / C, m2, mul, sub)
            nc.scalar.activation(
                var, var, mybir.ActivationFunctionType.Sqrt, bias=eps_sb, scale=1.0
            )
            rstd_f = statp.tile([P, T], F32, tag="stat", name="stat_t")
            nc.vector.reciprocal_approx_fast(rstd_f, var)
            rstd = statp.tile([P, T], BF16, tag="stat", name="stat_t")
            nc.vector.tensor_copy(rstd, rstd_f)
            S["m"], S["rstd"] = m_bf, rstd
            del S["muB"], S["sqB"]

        ch.append(c_statev)
        return ch

    def chunks_qkv(b):
        S = state[b]
        ch = []

        def c_xn(cc):
            if "xn" not in S:
                S["xn"] = xnpool.tile([P, CCH, T], BF16, tag="xn", name="xn_t")
            t = statp.tile([P, T], BF16, tag="stat", name="stat_t")
            nc.vector.tensor_tensor(t, S["x"][:, cc], S["m"], sub)
            nc.vector.tensor_tensor(S["xn"][:, cc], t, S["rstd"], mul)

        for cc in range(CCH):
            ch.append(lambda cc=cc: c_xn(cc))

        def c_dbg_a():
            if dbg is not None and b == 0:
                nc.sync.dma_start(dbg["stats"][0], S["m"])
                nc.sync.dma_start(dbg["stats"][1], S["rstd"])
                nc.sync.dma_start(dbg["xn"], S["xn"])

        ch.append(c_dbg_a)

        def c_qkgen(ot):
            if "qk" not in S:
                S["qk"] = qkpool.tile([P, 8, T], BF16, tag="qk", name="qk_t")
            ps = psp.tile([P, T], F32, tag="ps", name="ps_t")
            # cc-outer / half-inner: each weight chunk loads once
            for cc in range(CCH):
                for _, hs in HALVES:
                    nc.tensor.matmul(
                        ps[:, hs],
                        wqk_sb[:, cc, ot * P : (ot + 1) * P],
                        S["xn"][:, cc, hs],
                        start=(cc == 0), stop=(cc == CCH - 1),
                        skip_group_check=True,
                    )
            nc.vector.tensor_scalar_add(S["qk"][:, ot], ps, bqk_sb[:, ot : ot + 1])

        for ot in range(8):
            ch.append(lambda ot=ot: c_qkgen(ot))

        def c_vgen(st):
            ps = psp.tile([P, T], F32, tag="ps", name="ps_t")
            tsl = slice(st * P, (st + 1) * P)
            for cc in range(CCH):
                nc.tensor.matmul(
                    ps[:, 0:512],
                    S["xn"][:, cc, tsl],
                    wv_sb[:, cc, :],
                    start=(cc == 0), stop=(cc == CCH - 1),
                )
            pr = ps[:, 0:512].rearrange("p (h c) -> p h c", c=HD)
            bvr = bv_b.rearrange("p (h c) -> p h c", c=HD)
            v2r = v2_t[b].rearrange("p st (h c) -> p st h c", c=P)
            nc.vector.tensor_tensor(v2r[:, st, 0::2, 0:HD], pr[:, 0::2], bvr[:, 0::2], add)
            nc.vector.tensor_tensor(v2r[:, st, 1::2, HD:P], pr[:, 1::2], bvr[:, 1::2], add)

        for st in range(8):
            ch.append(lambda st=st: c_vgen(st))

        def c_dbg_b():
            if dbg is not None and b == 0:
                nc.sync.dma_start(dbg["qk"], S["qk"])
                nc.sync.dma_start(dbg["v2"], v2_t[b])

        ch.append(c_dbg_b)
        return ch

    # ---------------- phase B: attention ----------------
    def chunks_attn(b):
        S = state[b]
        ch = []

        def c_pair_start(pc):
            # acc[h01]: [128, 1024] = (64 data + 64 denom partitions) x
            # (half0 512q | half1 512q), one PSUM bank per half.
            S[("acc", pc)] = {
                h01: accp.tile([P, T], F32, tag="acc", name="acc_t") for h01 in (0, 1)
            }
            S[("rdb", pc)] = rdbp.tile([P, T], F32, tag="rdb", name="rdb_t")

        def c_st(pc, st):
            qt = S["qk"][:, 2 * pc]
            kt = S["qk"][:, 2 * pc + 1]
            tsl = slice(st * P, (st + 1) * P)
            es = {}
            for hf, hs in HALVES:
                pss = psp.tile([P, T], F32, tag="ps", name="ps_t")
                # row-tiled pair: head0 on PE rows 0:64 -> bank0,
                # head1 on rows 64:128 -> bank1; runs concurrently.
                for h01 in (0, 1):
                    bb = slice(HD * h01, HD * h01 + HD)
                    nc.tensor.matmul(
                        pss[:, 512 * h01 : 512 * h01 + 512],
                        kt[bb, tsl], qt[bb, hs],
                        start=True, stop=True,
                        tile_position=(HD * h01, 0),
                    )
                e = expp.tile([P, T], BF16, tag="exp", name="exp_t")
                nc.scalar.activation(e, pss, mybir.ActivationFunctionType.Exp, scale=0.125)
                es[hf] = e
                if dbg is not None and b == 0 and pc == 0 and st == 0 and hf == 0:
                    nc.sync.dma_start(dbg["exp"][0], e)
            # AV: stationary reused across halves per head
            for h01 in (0, 1):
                head = 2 * pc + h01
                for hf, hs in HALVES:
                    nc.tensor.matmul(
                        S[("acc", pc)][h01][:, 512 * hf : 512 * hf + 512],
                        v2_t[b][:, st, head * P : (head + 1) * P],
                        es[hf][:, 512 * h01 : 512 * h01 + 512],
                        start=(st == 0), stop=(st == 7),
                    )

        def c_fin(pc):
            if "h" not in S:
                S["h"] = hpool.tile([P, CCH, T], BF16, tag="h", name="h_t")
            rdb = S[("rdb", pc)]
            drow = drp.tile([2, T], F32, tag="drd", name="drd_t")
            rd_sb = rdsp.tile([P, T], F32, tag="rds", name="rds_t")
            hraw = rdsp.tile([P, T], BF16, tag="hraw", name="hraw_t")
            # evacuate raw d rows (DMA can't read PSUM): h0's denom lives at
            # partition 64, h1's at partition 0, into one tile
            for h01 in (0, 1):
                dn = HD * (1 - h01)
                nc.vector.tensor_copy(
                    rd_sb[dn : dn + 1, :], S[("acc", pc)][h01][dn : dn + 1, :]
                )
            # evacuate raw AV numerators so the PSUM banks free early; the
            # normalization multiply happens later when rdb lands.
            # h0 via ScalarE, h1 via VectorE (load balance).
            nc.scalar.activation(
                hraw[0:HD, :], S[("acc", pc)][0][0:HD, :],
                mybir.ActivationFunctionType.Copy,
            )
            nc.vector.tensor_copy(hraw[HD:P, :], S[("acc", pc)][1][HD:P, :])
            # bounce through DRAM to broadcast across the data partitions
            for h01 in (0, 1):
                dn = HD * (1 - h01)
                nc.sync.dma_start(
                    drow[(1 - h01) : (2 - h01), :], rd_sb[dn : dn + 1, :]
                )
            for h01 in (0, 1):
                d0 = HD * h01
                for q in (0, 1):  # split across DMA engines
                    bcast = bass.AP(
                        tensor=drow.tensor,
                        offset=drow.offset + (1 - h01) * T,
                        ap=[[0, HD // 2], [1, T]],
                    )
                    nc.sync.dma_start(rdb[d0 + q * 32 : d0 + q * 32 + 32, :], bcast)
            # one reciprocal for both heads; custom DVE op needs base
            # partition 0 (it silently breaks at base 64)
            nc.vector.reciprocal_approx_fast(rdb, rdb)
            for h01 in (0, 1):
                head = 2 * pc + h01
                d0 = HD * h01
                if dbg is not None and b == 0 and pc == 0:
                    nc.sync.dma_start(dbg["rdb"][h01], rdb[d0 : d0 + HD, :])
                cch = head // 2
                nc.vector.tensor_tensor(
                    S["h"][d0 : d0 + HD, cch, :],
                    hraw[d0 : d0 + HD, :],
                    rdb[d0 : d0 + HD, :],
                    mul,
                )

        for pc in range(4):
            ch.append(lambda pc=pc: c_pair_start(pc))
            for st in range(8):
                ch.append(lambda pc=pc, st=st: c_st(pc, st))
            ch.append(lambda pc=pc: c_fin(pc))
        return ch

    # ---------------- phase C: proj + residual + out ----------------
    def chunks_proj(b):
        S = state[b]
        ch = []

        def c_proj(ot):
            ps = psp.tile([P, T], F32, tag="ps", name="ps_t")
            for cc in range(CCH):
                for _, hs in HALVES:
                    nc.tensor.matmul(
                        ps[:, hs],
                        wp_sb[:, cc, ot * P : (ot + 1) * P],
                        S["h"][:, cc, hs],
                        start=(cc == 0), stop=(cc == CCH - 1),
                        skip_group_check=True,
                    )
            for hf, hs in HALVES:
                o_t = outp.tile([P, 512], F32, tag="out", name="out_t")
                nc.vector.scalar_tensor_tensor(
                    o_t, ps[:, hs], bp_sb[:, ot : ot + 1], S["x"][:, ot, hs], add, add
                )
                # split across DMA engines
                for q in (0, 1):
                    qs = slice(hs.start + q * 256, hs.start + q * 256 + 256)
                    nc.sync.dma_start(ov[b, :, ot, qs], o_t[:, q * 256 : q * 256 + 256])

        def c_dbg_h():
            if dbg is not None and b == 0:
                nc.sync.dma_start(dbg["h"], S["h"])

        ch.append(c_dbg_h)
        for ot in range(CCH):
            ch.append(lambda ot=ot: c_proj(ot))
        return ch

    # ---------------- emission schedule (software pipeline) ----------------
    l0, s0, q0 = chunks_load(0), chunks_stats(0), chunks_qkv(0)
    l1, s1, q1 = chunks_load(1), chunks_stats(1), chunks_qkv(1)
    l0[0]()
    emit_consts()
    for c in s0:
        c()
    l1[0]()
    for c in q0:
        c()
    # batch-1 stats (incl. its Sqrt) before the first exp: no activation
    # table swaps once attention starts.
    for c in s1:
        c()
    # batch-0 attention (Scalar-heavy) carries batch-1 QKV (PE-heavy)
    for c in _interleave(chunks_attn(0), q1):
        c()
    for c in _interleave(chunks_attn(1), chunks_proj(0)):
        c()
    for c in chunks_proj(1):
        c()


def build_nc(debug_taps=False):
    nc = bacc.Bacc("TRN2", num_devices=N_CORES, debug=False)
    x = nc.declare_dram_parameter("x", [B_LOC, C, T], BF16, isOutput=False)
    wqk = nc.declare_dram_parameter("w_qkT", [C, 2 * C], BF16, isOutput=False)
    wv = nc.declare_dram_parameter("w_vT", [C, C], BF16, isOutput=False)
    wp = nc.declare_dram_parameter("w_projT", [C, C], BF16, isOutput=False)
    bqk = nc.declare_dram_parameter("b_qk", [2 * C], F32, isOutput=False)
    bv = nc.declare_dram_parameter("b_v", [C], F32, isOutput=False)
    bp = nc.declare_dram_parameter("b_proj", [C], F32, isOutput=False)
    out = nc.declare_dram_parameter("out", [B_LOC, C, T], F32, isOutput=True)
    aps = (x.ap(), wqk.ap(), wv.ap(), wp.ap(), bqk.ap(), bv.ap(), bp.ap(), out.ap())
    dbg = None
    if debug_taps:
        dbg = {
            "stats": nc.declare_dram_parameter("dbg_stats", [2, P, T], BF16, isOutput=True).ap(),
            "xn": nc.declare_dram_parameter("dbg_xn", [P, CCH, T], BF16, isOutput=True).ap(),
            "qk": nc.declare_dram_parameter("dbg_qk", [P, 8, T], BF16, isOutput=True).ap(),
            "v2": nc.declare_dram_parameter("dbg_v2", [P, 8, N_HEADS * P], BF16, isOutput=True).ap(),
            "exp": nc.declare_dram_parameter("dbg_exp", [2, P, T], FP8, isOutput=True).ap(),
            "rdb": nc.declare_dram_parameter("dbg_rdb", [2, HD, T], F32, isOutput=True).ap(),
            "h": nc.declare_dram_parameter("dbg_h", [P, CCH, T], BF16, isOutput=True).ap(),
        }

    with tile.TileContext(nc) as tc:
        import contextlib

        with contextlib.ExitStack() as ctx:
            pools = (
                ctx.enter_context(tc.tile_pool(name="const", bufs=1)),
                ctx.enter_context(tc.tile_pool(name="x", bufs=2)),
                ctx.enter_context(tc.tile_pool(name="x2", bufs=1)),
                ctx.enter_context(tc.tile_pool(name="xn", bufs=2)),
                ctx.enter_context(tc.tile_pool(name="stat", bufs=5)),
                ctx.enter_context(tc.tile_pool(name="qk", bufs=2)),
                ctx.enter_context(tc.tile_pool(name="h", bufs=2)),
                ctx.enter_context(tc.tile_pool(name="exp", bufs=6)),
                ctx.enter_context(tc.tile_pool(name="rds", bufs=2)),
                ctx.enter_context(tc.tile_pool(name="rdb", bufs=2)),
                ctx.enter_context(tc.tile_pool(name="out", bufs=2)),
                ctx.enter_context(tc.tile_pool(name="ps", bufs=2, space="PSUM")),
                ctx.enter_context(tc.tile_pool(name="acc", bufs=2, space="PSUM")),
                ctx.enter_context(tc.tile_pool(name="drd", bufs=4, space="DRAM")),
            )
            _emit(tc, nc, pools, aps, dbg)
    nc.compile()
    return nc


def _host_prep(w_qkv, b_qkv, w_proj, b_proj):
    rows = np.arange(3 * C).reshape(N_HEADS, 3, HD)
    qk_order = []
    for pc in range(4):
        qk_order += list(rows[2 * pc, 0]) + list(rows[2 * pc + 1, 0])
        qk_order += list(rows[2 * pc, 1]) + list(rows[2 * pc + 1, 1])
    qk_order = np.array(qk_order)
    v_order = rows[:, 2, :].reshape(-1)
    prep = {
        "w_qkT": np.ascontiguousarray(w_qkv[qk_order].T).astype(ml_dtypes.bfloat16),
        "w_vT": np.ascontiguousarray(w_qkv[v_order].T).astype(ml_dtypes.bfloat16),
        "w_projT": np.ascontiguousarray(w_proj.T).astype(ml_dtypes.bfloat16),
        "b_qk": np.ascontiguousarray(b_qkv[qk_order]).astype(np.float32),
        "b_v": np.ascontiguousarray(b_qkv[v_order]).astype(np.float32),
        "b_proj": np.ascontiguousarray(b_proj).astype(np.float32),
    }
    return prep


def _make_in_maps(x, w_qkv, b_qkv, w_proj, b_proj):
    prep = _host_prep(
        np.asarray(w_qkv, np.float32), np.asarray(b_qkv, np.float32),
        np.asarray(w_proj, np.float32), np.asarray(b_proj, np.float32),
    )
    xf = np.asarray(x, np.float32).reshape(B, C, T).astype(ml_dtypes.bfloat16)
    in_maps = []
    for core in range(N_CORES):
        m = dict(prep)
        m["x"] = np.ascontiguousarray(xf[core * B_LOC : (core + 1) * B_LOC])
        in_maps.append(m)
    return in_maps


_NC = None


def kernel(x, emb, w_qkv, b_qkv, w_proj, b_proj):
    global _NC
    x = np.asarray(x, dtype=np.float32)
    b, c, hh, ww = x.shape
    assert (b, c, hh * ww) == (B, C, T)
    if _NC is None:
        _NC = build_nc()
    in_maps = _make_in_maps(x, w_qkv, b_qkv, w_proj, b_proj)
    res = run_bass_kernel_spmd(_NC, in_maps, core_ids=list(range(N_CORES)), trace=False)
    out = np.concatenate([res.results[i]["out"] for i in range(N_CORES)], axis=0)
    return out.reshape(B, C, hh, ww).astype(np.float32)


# revision 47
# speedup vs baseline: 1.0073x; 1.0073x over previous
"""Trainium2 Bass kernel for nn_AttentionBlock (B=16, C=512, H=W=32, 8 heads).

Data-parallel over batch: 16 batches / 8 cores = 2 per core.

v2 design (vs baseline):
  - x converted to bf16 on host: halves input DMA, removes bf16-staging
    copies on ScalarE, enables 2x DVE modes for LN elementwise ops.
  - S matmuls (K=64 per head) row-tiled: the two heads of a pair run on
    PE tiles (0,0)/(64,0) concurrently -> ~2x on the S phase.
  - One exp per (pair, st, half) over [128, 1024] PSUM (covers both heads).
  - Softmax denominator via the AV ones-columns trick, then ONE
    reciprocal per (head) [1, 1024], DRAM-bounce broadcast to 64
    partitions, single multiply per (head, half) for the h eviction.
    (Replaces baseline's per-(head,half) recip/copy/recip/mul chain.)
  - Both batches' LN stats (the only non-exp ScalarE table users) run
    before the first exp: zero activation-table swaps in steady state.
  - Stationary reuse: LN stats share one ones ldweights; QKV/proj
    accumulate cc-outer/half-inner so each weight chunk loads once.
  - v2 ones tiles persist across calls (memset once at start).

All matmuls bf16 (fp32 PSUM accumulation). I/O: x bf16 (host-cast),
out fp32.
"""

import math

import numpy as np
import ml_dtypes

import concourse.bass as bass
import concourse.bacc as bacc
import concourse.tile as tile
from concourse import mybir
from concourse.bass_utils import run_bass_kernel_spmd

P = 128
C = 512
T = 1024
N_HEADS = 8
HD = 64
B = 16
N_CORES = 8
B_LOC = B // N_CORES  # batches per core
CCH = C // P  # channel chunks of 128
EPS = 1e-5

F32 = mybir.dt.float32
BF16 = mybir.dt.bfloat16
FP8 = mybir.dt.float8e4
LN16 = math.log(16.0)

HALVES = ((0, slice(0, 512)), (1, slice(512, 1024)))


def _interleave(*seqs):
    """Proportional merge of chunk lists (stable within each list)."""
    items = []
    for si, s in enumerate(seqs):
        n = max(len(s), 1)
        for i, c in enumerate(s):
            items.append(((i + 0.5) / n, si, c))
    items.sort(key=lambda t: (t[0], t[1]))
    return [c for _, _, c in items]


def _emit(tc, nc, pools, aps, dbg=None):
    mul = mybir.AluOpType.mult
    add = mybir.AluOpType.add
    sub = mybir.AluOpType.subtract

    x_d, wqk_d, wv_d, wp_d, bqk_d, bv_d, bp_d, out_d = aps
    (const, xpool, x2pool, xnpool, statp, qkpool, hpool, expp, rdsp, rdbp, outp,
     psp, accp, drp) = pools

    # DRAM views
    xv = x_d.rearrange("b (cc p) t -> b p cc t", p=P)
    ov = out_d.rearrange("b (cc p) t -> b p cc t", p=P)

    # ---- persistent tiles ----
    wqk_sb = const.tile([P, CCH, 2 * C], BF16)
    wv_sb = const.tile([P, CCH, C], BF16)
    wp_sb = const.tile([P, CCH, C], BF16)
    bqk_sb = const.tile([P, 2 * C // P], F32)
    bp_sb = const.tile([P, CCH], F32)
    bv_b = const.tile([P, C], F32)
    ones_b = const.tile([P, P], BF16)
    eps_sb = const.tile([P, 1], F32)
    nln16_sb = const.tile([P, 1], F32)
    # per-batch v2 tiles: [t-chunk partitions, st, head*128 + (data|ones)]
    # even head: v data in cols 0:64 (ones in 64:128); odd head reversed.
    v2_t = [
        const.tile([P, 8, N_HEADS * P], BF16, name=f"v2_{b}") for b in range(B_LOC)
    ]

    def emit_consts():
        nc.vector.memset(ones_b, 1.0)
        nc.vector.memset(eps_sb, EPS)
        nc.vector.memset(nln16_sb, -LN16)
        for b in range(B_LOC):
            # gpsimd: slow but fully parallel to the DVE-heavy startup
            nc.gpsimd.memset(v2_t[b], 1.0)
        nc.sync.dma_start(wqk_sb, wqk_d.rearrange("(cc p) o -> p cc o", p=P))
        nc.sync.dma_start(wv_sb, wv_d.rearrange("(cc p) o -> p cc o", p=P))
        nc.sync.dma_start(bqk_sb, bqk_d.rearrange("(o p) -> p o", p=P))
        nc.sync.dma_start(
            bv_b,
            bass.AP(tensor=bv_d.tensor, offset=bv_d.offset, ap=[[0, P]] + list(bv_d.ap)),
        )
        nc.sync.dma_start(bp_sb, bp_d.rearrange("(o p) -> p o", p=P))
        nc.sync.dma_start(wp_sb, wp_d.rearrange("(cc p) o -> p cc o", p=P))

    state = [dict() for _ in range(B_LOC)]

    # ---------------- phase A: LN + QKV ----------------
    def chunks_load(b):
        S = state[b]

        def c_load():
            S["x"] = xpool.tile([P, CCH, T], BF16, tag="x", name="x_t")
            for cc in range(CCH):
                nc.sync.dma_start(S["x"][:, cc], xv[b, :, cc])

        return [c_load]

    def chunks_stats(b):
        S = state[b]
        ch = []

        def c_sq(cc):
            if "x2" not in S:
                S["x2"] = x2pool.tile([P, CCH, T], BF16, tag="x2", name="x2_t")
            nc.vector.tensor_tensor(S["x2"][:, cc], S["x"][:, cc], S["x"][:, cc], mul)

        for cc in range(CCH):
            ch.append(lambda cc=cc: c_sq(cc))

        def c_statmm():
            S["muB"] = psp.tile([P, T], F32, tag="ps", name="ps_t")
            S["sqB"] = psp.tile([P, T], F32, tag="ps", name="ps_t")
            # all 16 matmuls share the ones stationary
            for _, hs in HALVES:
                for cc in range(CCH):
                    nc.tensor.matmul(
                        S["muB"][:, hs], ones_b, S["x"][:, cc, hs],
                        start=(cc == 0), stop=(cc == CCH - 1),
                    )
            for _, hs in HALVES:
                for cc in range(CCH):
                    nc.tensor.matmul(
                        S["sqB"][:, hs], ones_b, S["x2"][:, cc, hs],
                        start=(cc == 0), stop=(cc == CCH - 1),
                    )

        ch.append(c_statmm)

        def c_statev():
            m_bf = statp.tile([P, T], BF16, tag="stat", name="stat_t")
            nc.vector.tensor_scalar_mul(m_bf, S["muB"], 1.0 / C)
            m2 = statp.tile([P, T], BF16, tag="stat", name="stat_t")
            nc.vector.tensor_tensor(m2, m_bf, m_bf, mul)
            var = statp.tile([P, T], F32, tag="stat", name="stat_t")
            nc.vector.scalar_tensor_tensor(var, S["sqB"], 1.0 / C, m2, mul, sub)
            nc.scalar.activation(
                var, var, mybir.ActivationFunctionType.Sqrt, bias=eps_sb, scale=1.0
            )
            rstd_f = statp.tile([P, T], F32, tag="stat", name="stat_t")
            nc.vector.reciprocal_approx_fast(rstd_f, var)
            rstd = statp.tile([P, T], BF16, tag="stat", name="stat_t")
            nc.vector.tensor_copy(rstd, rstd_f)
            S["m"], S["rstd"] = m_bf, rstd
            del S["muB"], S["sqB"]

        ch.append(c_statev)
        return ch

    def chunks_qkv(b):
        S = state[b]
        ch = []

        def c_xn(cc):
            if "xn" not in S:
                S["xn"] = xnpool.tile([P, CCH, T], BF16, tag="xn", name="xn_t")
            t = statp.tile([P, T], BF16, tag="stat", name="stat_t")
            nc.vector.tensor_tensor(t, S["x"][:, cc], S["m"], sub)
            nc.vector.tensor_tensor(S["xn"][:, cc], t, S["rstd"], mul)

        for cc in range(CCH):
            ch.append(lambda cc=cc: c_xn(cc))

        def c_dbg_a():
            if dbg is not None and b == 0:
                nc.sync.dma_start(dbg["stats"][0], S["m"])
                nc.sync.dma_start(dbg["stats"][1], S["rstd"])
                nc.sync.dma_start(dbg["xn"], S["xn"])

        ch.append(c_dbg_a)

        def c_qkgen(ot):
            if "qk" not in S:
                S["qk"] = qkpool.tile([P, 8, T], BF16, tag="qk", name="qk_t")
            ps = psp.tile([P, T], F32, tag="ps", name="ps_t")
            # cc-outer / half-inner: each weight chunk loads once
            for cc in range(CCH):
                for _, hs in HALVES:
                    nc.tensor.matmul(
                        ps[:, hs],
                        wqk_sb[:, cc, ot * P : (ot + 1) * P],
                        S["xn"][:, cc, hs],
                        start=(cc == 0), stop=(cc == CCH - 1),
                        skip_group_check=True,
                    )
            nc.vector.tensor_scalar_add(S["qk"][:, ot], ps, bqk_sb[:, ot : ot + 1])

        for ot in range(8):
            ch.append(lambda ot=ot: c_qkgen(ot))

        def c_vgen(st):
            ps = psp.tile([P, T], F32, tag="ps", name="ps_t")
            tsl = slice(st * P, (st + 1) * P)
            for cc in range(CCH):
                nc.tensor.matmul(
                    ps[:, 0:512],
                    S["xn"][:, cc, tsl],
                    wv_sb[:, cc, :],
                    start=(cc == 0), stop=(cc == CCH - 1),
                )
            pr = ps[:, 0:512].rearrange("p (h c) -> p h c", c=HD)
            bvr = bv_b.rearrange("p (h c) -> p h c", c=HD)
            v2r = v2_t[b].rearrange("p st (h c) -> p st h c", c=P)
            nc.vector.tensor_tensor(v2r[:, st, 0::2, 0:HD], pr[:, 0::2], bvr[:, 0::2], add)
            nc.vector.tensor_tensor(v2r[:, st, 1::2, HD:P], pr[:, 1::2], bvr[:, 1::2], add)

        for st in range(8):
            ch.append(lambda st=st: c_vgen(st))

        def c_dbg_b():
            if dbg is not None and b == 0:
                nc.sync.dma_start(dbg["qk"], S["qk"])
                nc.sync.dma_start(dbg["v2"], v2_t[b])

        ch.append(c_dbg_b)
        return ch

    # ---------------- phase B: attention ----------------
    def chunks_attn(b):
        S = state[b]
        ch = []

        def c_pair_start(pc):
            # acc[h01]: [128, 1024] = (64 data + 64 denom partitions) x
            # (half0 512q | half1 512q), one PSUM bank per half.
            S[("acc", pc)] = {
                h01: accp.tile([P, T], F32, tag="acc", name="acc_t") for h01 in (0, 1)
            }
            S[("rdb", pc)] = rdbp.tile([P, T], F32, tag="rdb", name="rdb_t")

        def c_st(pc, st):
            qt = S["qk"][:, 2 * pc]
            kt = S["qk"][:, 2 * pc + 1]
            tsl = slice(st * P, (st + 1) * P)
            es = {}
            for hf, hs in HALVES:
                pss = psp.tile([P, T], F32, tag="ps", name="ps_t")
                # row-tiled pair: head0 on PE rows 0:64 -> bank0,
                # head1 on rows 64:128 -> bank1; runs concurrently.
                for h01 in (0, 1):
                    bb = slice(HD * h01, HD * h01 + HD)
                    nc.tensor.matmul(
                        pss[:, 512 * h01 : 512 * h01 + 512],
                        kt[bb, tsl], qt[bb, hs],
                        start=True, stop=True,
                        tile_position=(HD * h01, 0),
                    )
                e = expp.tile([P, T], BF16, tag="exp", name="exp_t")
                nc.scalar.activation(e, pss, mybir.ActivationFunctionType.Exp, scale=0.125)
                es[hf] = e
                if dbg is not None and b == 0 and pc == 0 and st == 0 and hf == 0:
                    nc.sync.dma_start(dbg["exp"][0], e)
            # AV: stationary reused across halves per head
            for h01 in (0, 1):
                head = 2 * pc + h01
                for hf, hs in HALVES:
                    nc.tensor.matmul(
                        S[("acc", pc)][h01][:, 512 * hf : 512 * hf + 512],
                        v2_t[b][:, st, head * P : (head + 1) * P],
                        es[hf][:, 512 * h01 : 512 * h01 + 512],
                        start=(st == 0), stop=(st == 7),
                    )

        def c_fin(pc):
            if "h" not in S:
                S["h"] = hpool.tile([P, CCH, T], BF16, tag="h", name="h_t")
            rdb = S[("rdb", pc)]
            drow = drp.tile([2, T], F32, tag="drd", name="drd_t")
            rd_sb = rdsp.tile([P, T], F32, tag="rds", name="rds_t")
            hraw = rdsp.tile([P, T], BF16, tag="hraw", name="hraw_t")
            # evacuate raw d rows (DMA can't read PSUM): h0's denom lives at
            # partition 64, h1's at partition 0, into one tile
            for h01 in (0, 1):
                dn = HD * (1 - h01)
                nc.vector.tensor_copy(
                    rd_sb[dn : dn + 1, :], S[("acc", pc)][h01][dn : dn + 1, :]
                )
            # evacuate raw AV numerators so the PSUM banks free early; the
            # normalization multiply happens later when rdb lands.
            # h0 via ScalarE, h1 via VectorE (load balance).
            nc.scalar.activation(
                hraw[0:HD, :], S[("acc", pc)][0][0:HD, :],
                mybir.ActivationFunctionType.Copy,
            )
            nc.vector.tensor_copy(hraw[HD:P, :], S[("acc", pc)][1][HD:P, :])
            # bounce through DRAM to broadcast across the data partitions
            for h01 in (0, 1):
                dn = HD * (1 - h01)
                nc.sync.dma_start(
                    drow[(1 - h01) : (2 - h01), :], rd_sb[dn : dn + 1, :]
                )
            for h01 in (0, 1):
                d0 = HD * h01
                for q in (0, 1):  # split across trigger queues + DMA engines
                    bcast = bass.AP(
                        tensor=drow.tensor,
                        offset=drow.offset + (1 - h01) * T,
                        ap=[[0, HD // 2], [1, T]],
                    )
                    eng = nc.gpsimd if q == 0 else nc.sync
                    eng.dma_start(rdb[d0 + q * 32 : d0 + q * 32 + 32, :], bcast)
            # one reciprocal for both heads; custom DVE op needs base
            # partition 0 (it silently breaks at base 64)
            nc.vector.reciprocal_approx_fast(rdb, rdb)
            for h01 in (0, 1):
                head = 2 * pc + h01
                d0 = HD * h01
                if dbg is not None and b == 0 and pc == 0:
                    nc.sync.dma_start(dbg["rdb"][h01], rdb[d0 : d0 + HD, :])
                cch = head // 2
                nc.vector.tensor_tensor(
                    S["h"][d0 : d0 + HD, cch, :],
                    hraw[d0 : d0 + HD, :],
                    rdb[d0 : d0 + HD, :],
                    mul,
                )

        for pc in range(4):
            ch.append(lambda pc=pc: c_pair_start(pc))
            for st in range(8):
                ch.append(lambda pc=pc, st=st: c_st(pc, st))
            ch.append(lambda pc=pc: c_fin(pc))
        return ch

    # ---------------- phase C: proj + residual + out ----------------
    def chunks_proj(b):
        S = state[b]
        ch = []

        def c_proj(ot):
            ps = psp.tile([P, T], F32, tag="ps", name="ps_t")
            for cc in range(CCH):
                for _, hs in HALVES:
                    nc.tensor.matmul(
                        ps[:, hs],
                        wp_sb[:, cc, ot * P : (ot + 1) * P],
                        S["h"][:, cc, hs],
                        start=(cc == 0), stop=(cc == CCH - 1),
                        skip_group_check=True,
                    )
            for hf, hs in HALVES:
                o_t = outp.tile([P, 512], F32, tag="out", name="out_t")
                nc.vector.scalar_tensor_tensor(
                    o_t, ps[:, hs], bp_sb[:, ot : ot + 1], S["x"][:, ot, hs], add, add
                )
                nc.sync.dma_start(ov[b, :, ot, hs], o_t)

        def c_dbg_h():
            if dbg is not None and b == 0:
                nc.sync.dma_start(dbg["h"], S["h"])

        ch.append(c_dbg_h)
        for ot in range(CCH):
            ch.append(lambda ot=ot: c_proj(ot))
        return ch

    # ---------------- emission schedule (software pipeline) ----------------
    l0, s0, q0 = chunks_load(0), chunks_stats(0), chunks_qkv(0)
    l1, s1, q1 = chunks_load(1), chunks_stats(1), chunks_qkv(1)
    l0[0]()
    emit_consts()
    for c in s0:
        c()
    l1[0]()
    for c in q0:
        c()
    # batch-1 stats (incl. its Sqrt) before the first exp: no activation
    # table swaps once attention starts.
    for c in s1:
        c()
    # batch-0 attention (Scalar-heavy) carries batch-1 QKV (PE-heavy)
    for c in _interleave(chunks_attn(0), q1):
        c()
    for c in _interleave(chunks_attn(1), chunks_proj(0)):
        c()
    for c in chunks_proj(1):
        c()


def build_nc(debug_taps=False):
    nc = bacc.Bacc("TRN2", num_devices=N_CORES, debug=False)
    x = nc.declare_dram_parameter("x", [B_LOC, C, T], BF16, isOutput=False)
    wqk = nc.declare_dram_parameter("w_qkT", [C, 2 * C], BF16, isOutput=False)
    wv = nc.declare_dram_parameter("w_vT", [C, C], BF16, isOutput=False)
    wp = nc.declare_dram_parameter("w_projT", [C, C], BF16, isOutput=False)
    bqk = nc.declare_dram_parameter("b_qk", [2 * C], F32, isOutput=False)
    bv = nc.declare_dram_parameter("b_v", [C], F32, isOutput=False)
    bp = nc.declare_dram_parameter("b_proj", [C], F32, isOutput=False)
    out = nc.declare_dram_parameter("out", [B_LOC, C, T], F32, isOutput=True)
    aps = (x.ap(), wqk.ap(), wv.ap(), wp.ap(), bqk.ap(), bv.ap(), bp.ap(), out.ap())
    dbg = None
    if debug_taps:
        dbg = {
            "stats": nc.declare_dram_parameter("dbg_stats", [2, P, T], BF16, isOutput=True).ap(),
            "xn": nc.declare_dram_parameter("dbg_xn", [P, CCH, T], BF16, isOutput=True).ap(),
            "qk": nc.declare_dram_parameter("dbg_qk", [P, 8, T], BF16, isOutput=True).ap(),
            "v2": nc.declare_dram_parameter("dbg_v2", [P, 8, N_HEADS * P], BF16, isOutput=True).ap(),
            "exp": nc.declare_dram_parameter("dbg_exp", [2, P, T], FP8, isOutput=True).ap(),
            "rdb": nc.declare_dram_parameter("dbg_rdb", [2, HD, T], F32, isOutput=True).ap(),
            "h": nc.declare_dram_parameter("dbg_h", [P, CCH, T], BF16, isOutput=True).ap(),
        }

    with tile.TileContext(nc) as tc:
        import contextlib

        with contextlib.ExitStack() as ctx:
            pools = (
                ctx.enter_context(tc.tile_pool(name="const", bufs=1)),
                ctx.enter_context(tc.tile_pool(name="x", bufs=2)),
                ctx.enter_context(tc.tile_pool(name="x2", bufs=1)),
                ctx.enter_context(tc.tile_pool(name="xn", bufs=2)),
                ctx.enter_context(tc.tile_pool(name="stat", bufs=5)),
                ctx.enter_context(tc.tile_pool(name="qk", bufs=2)),
                ctx.enter_context(tc.tile_pool(name="h", bufs=2)),
                ctx.enter_context(tc.tile_pool(name="exp", bufs=4)),
                ctx.enter_context(tc.tile_pool(name="rds", bufs=2)),
                ctx.enter_context(tc.tile_pool(name="rdb", bufs=2)),
                ctx.enter_context(tc.tile_pool(name="out", bufs=2)),
                ctx.enter_context(tc.tile_pool(name="ps", bufs=2, space="PSUM")),
                ctx.enter_context(tc.tile_pool(name="acc", bufs=2, space="PSUM")),
                ctx.enter_context(tc.tile_pool(name="drd", bufs=4, space="DRAM")),
            )
            _emit(tc, nc, pools, aps, dbg)
    nc.compile()
    return nc


def _host_prep(w_qkv, b_qkv, w_proj, b_proj):
    rows = np.arange(3 * C).reshape(N_HEADS, 3, HD)
    qk_order = []
    for pc in range(4):
        qk_order += list(rows[2 * pc, 0]) + list(rows[2 * pc + 1, 0])
        qk_order += list(rows[2 * pc, 1]) + list(rows[2 * pc + 1, 1])
    qk_order = np.array(qk_order)
    v_order = rows[:, 2, :].reshape(-1)
    prep = {
        "w_qkT": np.ascontiguousarray(w_qkv[qk_order].T).astype(ml_dtypes.bfloat16),
        "w_vT": np.ascontiguousarray(w_qkv[v_order].T).astype(ml_dtypes.bfloat16),
        "w_projT": np.ascontiguousarray(w_proj.T).astype(ml_dtypes.bfloat16),
        "b_qk": np.ascontiguousarray(b_qkv[qk_order]).astype(np.float32),
        "b_v": np.ascontiguousarray(b_qkv[v_order]).astype(np.float32),
        "b_proj": np.ascontiguousarray(b_proj).astype(np.float32),
    }
    return prep


def _make_in_maps(x, w_qkv, b_qkv, w_proj, b_proj):
    prep = _host_prep(
        np.asarray(w_qkv, np.float32), np.asarray(b_qkv, np.float32),
        np.asarray(w_proj, np.float32), np.asarray(b_proj, np.float32),
    )
    xf = np.asarray(x, np.float32).reshape(B, C, T).astype(ml_dtypes.bfloat16)
    in_maps = []
    for core in range(N_CORES):
        m = dict(prep)
        m["x"] = np.ascontiguousarray(xf[core * B_LOC : (core + 1) * B_LOC])
        in_maps.append(m)
    return in_maps


_NC = None


def kernel(x, emb, w_qkv, b_qkv, w_proj, b_proj):
    global _NC
    x = np.asarray(x, dtype=np.float32)
    b, c, hh, ww = x.shape
    assert (b, c, hh * ww) == (B, C, T)
    if _NC is None:
        _NC = build_nc()
    in_maps = _make_in_maps(x, w_qkv, b_qkv, w_proj, b_proj)
    res = run_bass_kernel_spmd(_NC, in_maps, core_ids=list(range(N_CORES)), trace=False)
    out = np.concatenate([res.results[i]["out"] for i in range(N_CORES)], axis=0)
    return out.reshape(B, C, hh, ww).astype(np.float32)


# revision 48
# speedup vs baseline: 1.0140x; 1.0067x over previous
"""Trainium2 Bass kernel for nn_AttentionBlock (B=16, C=512, H=W=32, 8 heads).

Data-parallel over batch: 16 batches / 8 cores = 2 per core.

v2 design (vs baseline):
  - x converted to bf16 on host: halves input DMA, removes bf16-staging
    copies on ScalarE, enables 2x DVE modes for LN elementwise ops.
  - S matmuls (K=64 per head) row-tiled: the two heads of a pair run on
    PE tiles (0,0)/(64,0) concurrently -> ~2x on the S phase.
  - One exp per (pair, st, half) over [128, 1024] PSUM (covers both heads).
  - Softmax denominator via the AV ones-columns trick, then ONE
    reciprocal per (head) [1, 1024], DRAM-bounce broadcast to 64
    partitions, single multiply per (head, half) for the h eviction.
    (Replaces baseline's per-(head,half) recip/copy/recip/mul chain.)
  - Both batches' LN stats (the only non-exp ScalarE table users) run
    before the first exp: zero activation-table swaps in steady state.
  - Stationary reuse: LN stats share one ones ldweights; QKV/proj
    accumulate cc-outer/half-inner so each weight chunk loads once.
  - v2 ones tiles persist across calls (memset once at start).

All matmuls bf16 (fp32 PSUM accumulation). I/O: x bf16 (host-cast),
out fp32.
"""

import math

import numpy as np
import ml_dtypes

import concourse.bass as bass
import concourse.bacc as bacc
import concourse.tile as tile
from concourse import mybir
from concourse.bass_utils import run_bass_kernel_spmd

P = 128
C = 512
T = 1024
N_HEADS = 8
HD = 64
B = 16
N_CORES = 8
B_LOC = B // N_CORES  # batches per core
CCH = C // P  # channel chunks of 128
EPS = 1e-5

F32 = mybir.dt.float32
BF16 = mybir.dt.bfloat16
FP8 = mybir.dt.float8e4
LN16 = math.log(16.0)

HALVES = ((0, slice(0, 512)), (1, slice(512, 1024)))


def _interleave(*seqs):
    """Proportional merge of chunk lists (stable within each list)."""
    items = []
    for si, s in enumerate(seqs):
        n = max(len(s), 1)
        for i, c in enumerate(s):
            items.append(((i + 0.5) / n, si, c))
    items.sort(key=lambda t: (t[0], t[1]))
    return [c for _, _, c in items]


def _emit(tc, nc, pools, aps, dbg=None):
    mul = mybir.AluOpType.mult
    add = mybir.AluOpType.add
    sub = mybir.AluOpType.subtract

    x_d, wqk_d, wv_d, wp_d, bqk_d, bv_d, bp_d, out_d = aps
    (const, xpool, x2pool, xnpool, statp, qkpool, hpool, expp, rdsp, rdbp, outp,
     psp, accp, drp) = pools

    # DRAM views
    xv = x_d.rearrange("b (cc p) t -> b p cc t", p=P)
    ov = out_d.rearrange("b (cc p) t -> b p cc t", p=P)

    # ---- persistent tiles ----
    wqk_sb = const.tile([P, CCH, 2 * C], BF16)
    wv_sb = const.tile([P, CCH, C], BF16)
    wp_sb = const.tile([P, CCH, C], BF16)
    bqk_sb = const.tile([P, 2 * C // P], F32)
    bp_sb = const.tile([P, CCH], F32)
    bv_b = const.tile([P, C], F32)
    ones_b = const.tile([P, P], BF16)
    eps_sb = const.tile([P, 1], F32)
    nln16_sb = const.tile([P, 1], F32)
    # per-batch v2 tiles: [t-chunk partitions, st, head*128 + (data|ones)]
    # even head: v data in cols 0:64 (ones in 64:128); odd head reversed.
    v2_t = [
        const.tile([P, 8, N_HEADS * P], BF16, name=f"v2_{b}") for b in range(B_LOC)
    ]

    def emit_consts():
        nc.vector.memset(ones_b, 1.0)
        nc.vector.memset(eps_sb, EPS)
        nc.vector.memset(nln16_sb, -LN16)
        for b in range(B_LOC):
            # gpsimd: slow but fully parallel to the DVE-heavy startup
            nc.gpsimd.memset(v2_t[b], 1.0)
        nc.sync.dma_start(wqk_sb, wqk_d.rearrange("(cc p) o -> p cc o", p=P))
        nc.sync.dma_start(wv_sb, wv_d.rearrange("(cc p) o -> p cc o", p=P))
        nc.sync.dma_start(bqk_sb, bqk_d.rearrange("(o p) -> p o", p=P))
        nc.sync.dma_start(
            bv_b,
            bass.AP(tensor=bv_d.tensor, offset=bv_d.offset, ap=[[0, P]] + list(bv_d.ap)),
        )
        nc.sync.dma_start(bp_sb, bp_d.rearrange("(o p) -> p o", p=P))
        nc.sync.dma_start(wp_sb, wp_d.rearrange("(cc p) o -> p cc o", p=P))

    state = [dict() for _ in range(B_LOC)]

    # ---------------- phase A: LN + QKV ----------------
    def chunks_load(b):
        S = state[b]

        def c_load():
            S["x"] = xpool.tile([P, CCH, T], BF16, tag="x", name="x_t")
            for cc in range(CCH):
                nc.sync.dma_start(S["x"][:, cc], xv[b, :, cc])

        return [c_load]

    def chunks_stats(b):
        S = state[b]
        ch = []

        def c_sq(cc):
            if "x2" not in S:
                S["x2"] = x2pool.tile([P, CCH, T], BF16, tag="x2", name="x2_t")
            nc.vector.tensor_tensor(S["x2"][:, cc], S["x"][:, cc], S["x"][:, cc], mul)

        for cc in range(CCH):
            ch.append(lambda cc=cc: c_sq(cc))

        def c_statmm():
            S["muB"] = psp.tile([P, T], F32, tag="ps", name="ps_t")
            S["sqB"] = psp.tile([P, T], F32, tag="ps", name="ps_t")
            # all 16 matmuls share the ones stationary
            for _, hs in HALVES:
                for cc in range(CCH):
                    nc.tensor.matmul(
                        S["muB"][:, hs], ones_b, S["x"][:, cc, hs],
                        start=(cc == 0), stop=(cc == CCH - 1),
                    )
            for _, hs in HALVES:
                for cc in range(CCH):
                    nc.tensor.matmul(
                        S["sqB"][:, hs], ones_b, S["x2"][:, cc, hs],
                        start=(cc == 0), stop=(cc == CCH - 1),
                    )

        ch.append(c_statmm)

        def c_statev():
            m_bf = statp.tile([P, T], BF16, tag="stat", name="stat_t")
            nc.vector.tensor_scalar_mul(m_bf, S["muB"], 1.0 / C)
            m2 = statp.tile([P, T], BF16, tag="stat", name="stat_t")
            nc.vector.tensor_tensor(m2, m_bf, m_bf, mul)
            var = statp.tile([P, T], F32, tag="stat", name="stat_t")
            nc.vector.scalar_tensor_tensor(var, S["sqB"], 1.0 / C, m2, mul, sub)
            nc.scalar.activation(
                var, var, mybir.ActivationFunctionType.Sqrt, bias=eps_sb, scale=1.0
            )
            rstd_f = statp.tile([P, T], F32, tag="stat", name="stat_t")
            nc.vector.reciprocal_approx_fast(rstd_f, var)
            rstd = statp.tile([P, T], BF16, tag="stat", name="stat_t")
            nc.vector.tensor_copy(rstd, rstd_f)
            S["m"], S["rstd"] = m_bf, rstd
            del S["muB"], S["sqB"]

        ch.append(c_statev)
        return ch

    def chunks_qkv(b):
        S = state[b]
        ch = []

        def c_xn(cc):
            if "xn" not in S:
                S["xn"] = xnpool.tile([P, CCH, T], BF16, tag="xn", name="xn_t")
            t = statp.tile([P, T], BF16, tag="stat", name="stat_t")
            nc.vector.tensor_tensor(t, S["x"][:, cc], S["m"], sub)
            nc.vector.tensor_tensor(S["xn"][:, cc], t, S["rstd"], mul)

        for cc in range(CCH):
            ch.append(lambda cc=cc: c_xn(cc))

        def c_dbg_a():
            if dbg is not None and b == 0:
                nc.sync.dma_start(dbg["stats"][0], S["m"])
                nc.sync.dma_start(dbg["stats"][1], S["rstd"])
                nc.sync.dma_start(dbg["xn"], S["xn"])

        ch.append(c_dbg_a)

        def c_qkgen(ot):
            if "qk" not in S:
                S["qk"] = qkpool.tile([P, 8, T], BF16, tag="qk", name="qk_t")
            ps = psp.tile([P, T], F32, tag="ps", name="ps_t")
            # cc-outer / half-inner: each weight chunk loads once
            for cc in range(CCH):
                for _, hs in HALVES:
                    nc.tensor.matmul(
                        ps[:, hs],
                        wqk_sb[:, cc, ot * P : (ot + 1) * P],
                        S["xn"][:, cc, hs],
                        start=(cc == 0), stop=(cc == CCH - 1),
                        skip_group_check=True,
                    )
            nc.vector.tensor_scalar_add(S["qk"][:, ot], ps, bqk_sb[:, ot : ot + 1])

        for ot in range(8):
            ch.append(lambda ot=ot: c_qkgen(ot))

        def c_vgen(st):
            ps = psp.tile([P, T], F32, tag="ps", name="ps_t")
            tsl = slice(st * P, (st + 1) * P)
            for cc in range(CCH):
                nc.tensor.matmul(
                    ps[:, 0:512],
                    S["xn"][:, cc, tsl],
                    wv_sb[:, cc, :],
                    start=(cc == 0), stop=(cc == CCH - 1),
                )
            pr = ps[:, 0:512].rearrange("p (h c) -> p h c", c=HD)
            bvr = bv_b.rearrange("p (h c) -> p h c", c=HD)
            v2r = v2_t[b].rearrange("p st (h c) -> p st h c", c=P)
            nc.vector.tensor_tensor(v2r[:, st, 0::2, 0:HD], pr[:, 0::2], bvr[:, 0::2], add)
            nc.vector.tensor_tensor(v2r[:, st, 1::2, HD:P], pr[:, 1::2], bvr[:, 1::2], add)

        for st in range(8):
            ch.append(lambda st=st: c_vgen(st))

        def c_dbg_b():
            if dbg is not None and b == 0:
                nc.sync.dma_start(dbg["qk"], S["qk"])
                nc.sync.dma_start(dbg["v2"], v2_t[b])

        ch.append(c_dbg_b)
        return ch

    # ---------------- phase B: attention ----------------
    def chunks_attn(b):
        S = state[b]
        ch = []

        def c_pair_start(pc):
            # acc[h01]: [128, 1024] = (64 data + 64 denom partitions) x
            # (half0 512q | half1 512q), one PSUM bank per half.
            S[("acc", pc)] = {
                h01: accp.tile([P, T], F32, tag="acc", name="acc_t") for h01 in (0, 1)
            }
            S[("rdb", pc)] = rdbp.tile([P, T], F32, tag="rdb", name="rdb_t")

        def c_st(pc, st):
            qt = S["qk"][:, 2 * pc]
            kt = S["qk"][:, 2 * pc + 1]
            tsl = slice(st * P, (st + 1) * P)
            es = {}
            for hf, hs in HALVES:
                pss = psp.tile([P, T], F32, tag="ps", name="ps_t")
                # row-tiled pair: head0 on PE rows 0:64 -> bank0,
                # head1 on rows 64:128 -> bank1; runs concurrently.
                for h01 in (0, 1):
                    bb = slice(HD * h01, HD * h01 + HD)
                    nc.tensor.matmul(
                        pss[:, 512 * h01 : 512 * h01 + 512],
                        kt[bb, tsl], qt[bb, hs],
                        start=True, stop=True,
                        tile_position=(HD * h01, 0),
                    )
                e = expp.tile([P, T], BF16, tag="exp", name="exp_t")
                nc.scalar.activation(e, pss, mybir.ActivationFunctionType.Exp, scale=0.125)
                es[hf] = e
                if dbg is not None and b == 0 and pc == 0 and st == 0 and hf == 0:
                    nc.sync.dma_start(dbg["exp"][0], e)
            # AV: stationary reused across halves per head
            for h01 in (0, 1):
                head = 2 * pc + h01
                for hf, hs in HALVES:
                    nc.tensor.matmul(
                        S[("acc", pc)][h01][:, 512 * hf : 512 * hf + 512],
                        v2_t[b][:, st, head * P : (head + 1) * P],
                        es[hf][:, 512 * h01 : 512 * h01 + 512],
                        start=(st == 0), stop=(st == 7),
                    )

        def c_fin(pc):
            if "h" not in S:
                S["h"] = hpool.tile([P, CCH, T], BF16, tag="h", name="h_t")
            rdb = S[("rdb", pc)]
            drow = drp.tile([2, T], F32, tag="drd", name="drd_t")
            rd_sb = rdsp.tile([P, T], F32, tag="rds", name="rds_t")
            hraw = rdsp.tile([P, T], BF16, tag="hraw", name="hraw_t")
            # evacuate raw d rows (DMA can't read PSUM): h0's denom lives at
            # partition 64, h1's at partition 0, into one tile
            for h01 in (0, 1):
                dn = HD * (1 - h01)
                nc.vector.tensor_copy(
                    rd_sb[dn : dn + 1, :], S[("acc", pc)][h01][dn : dn + 1, :]
                )
            # evacuate raw AV numerators so the PSUM banks free early; the
            # normalization multiply happens later when rdb lands.
            # h0 via ScalarE, h1 via VectorE (load balance).
            nc.scalar.activation(
                hraw[0:HD, :], S[("acc", pc)][0][0:HD, :],
                mybir.ActivationFunctionType.Copy,
            )
            nc.vector.tensor_copy(hraw[HD:P, :], S[("acc", pc)][1][HD:P, :])
            # bounce through DRAM to broadcast across the data partitions
            for h01 in (0, 1):
                dn = HD * (1 - h01)
                nc.sync.dma_start(
                    drow[(1 - h01) : (2 - h01), :], rd_sb[dn : dn + 1, :]
                )
            for h01 in (0, 1):
                d0 = HD * h01
                for q in (0, 1):  # split across trigger queues + DMA engines
                    bcast = bass.AP(
                        tensor=drow.tensor,
                        offset=drow.offset + (1 - h01) * T,
                        ap=[[0, HD // 2], [1, T]],
                    )
                    eng = nc.gpsimd if q == 0 else nc.sync
                    eng.dma_start(rdb[d0 + q * 32 : d0 + q * 32 + 32, :], bcast)
            # one reciprocal for both heads; custom DVE op needs base
            # partition 0 (it silently breaks at base 64)
            nc.vector.reciprocal_approx_fast(rdb, rdb)
            for h01 in (0, 1):
                head = 2 * pc + h01
                d0 = HD * h01
                if dbg is not None and b == 0 and pc == 0:
                    nc.sync.dma_start(dbg["rdb"][h01], rdb[d0 : d0 + HD, :])
                cch = head // 2
                nc.vector.tensor_tensor(
                    S["h"][d0 : d0 + HD, cch, :],
                    hraw[d0 : d0 + HD, :],
                    rdb[d0 : d0 + HD, :],
                    mul,
                )

        for pc in range(4):
            ch.append(lambda pc=pc: c_pair_start(pc))
            for st in range(8):
                ch.append(lambda pc=pc, st=st: c_st(pc, st))
            ch.append(lambda pc=pc: c_fin(pc))
        return ch

    # ---------------- phase C: proj + residual + out ----------------
    def chunks_proj(b):
        S = state[b]
        ch = []

        def c_proj(ot):
            ps = psp.tile([P, T], F32, tag="ps", name="ps_t")
            for cc in range(CCH):
                for _, hs in HALVES:
                    nc.tensor.matmul(
                        ps[:, hs],
                        wp_sb[:, cc, ot * P : (ot + 1) * P],
                        S["h"][:, cc, hs],
                        start=(cc == 0), stop=(cc == CCH - 1),
                        skip_group_check=True,
                    )
            for hf, hs in HALVES:
                o_t = outp.tile([P, 512], F32, tag="out", name="out_t")
                nc.vector.scalar_tensor_tensor(
                    o_t, ps[:, hs], bp_sb[:, ot : ot + 1], S["x"][:, ot, hs], add, add
                )
                # split across DMA engines: halves the last-transfer tail
                for q in (0, 1):
                    qs = slice(hs.start + q * 256, hs.start + q * 256 + 256)
                    nc.sync.dma_start(ov[b, :, ot, qs], o_t[:, q * 256 : q * 256 + 256])

        def c_dbg_h():
            if dbg is not None and b == 0:
                nc.sync.dma_start(dbg["h"], S["h"])

        ch.append(c_dbg_h)
        for ot in range(CCH):
            ch.append(lambda ot=ot: c_proj(ot))
        return ch

    # ---------------- emission schedule (software pipeline) ----------------
    l0, s0, q0 = chunks_load(0), chunks_stats(0), chunks_qkv(0)
    l1, s1, q1 = chunks_load(1), chunks_stats(1), chunks_qkv(1)
    l0[0]()
    emit_consts()
    for c in s0:
        c()
    l1[0]()
    for c in q0:
        c()
    # batch-1 stats (incl. its Sqrt) before the first exp: no activation
    # table swaps once attention starts.
    for c in s1:
        c()
    # batch-0 attention (Scalar-heavy) carries batch-1 QKV (PE-heavy)
    for c in _interleave(chunks_attn(0), q1):
        c()
    for c in _interleave(chunks_attn(1), chunks_proj(0)):
        c()
    for c in chunks_proj(1):
        c()


def build_nc(debug_taps=False):
    nc = bacc.Bacc("TRN2", num_devices=N_CORES, debug=False)
    x = nc.declare_dram_parameter("x", [B_LOC, C, T], BF16, isOutput=False)
    wqk = nc.declare_dram_parameter("w_qkT", [C, 2 * C], BF16, isOutput=False)
    wv = nc.declare_dram_parameter("w_vT", [C, C], BF16, isOutput=False)
    wp = nc.declare_dram_parameter("w_projT", [C, C], BF16, isOutput=False)
    bqk = nc.declare_dram_parameter("b_qk", [2 * C], F32, isOutput=False)
    bv = nc.declare_dram_parameter("b_v", [C], F32, isOutput=False)
    bp = nc.declare_dram_parameter("b_proj", [C], F32, isOutput=False)
    out = nc.declare_dram_parameter("out", [B_LOC, C, T], F32, isOutput=True)
    aps = (x.ap(), wqk.ap(), wv.ap(), wp.ap(), bqk.ap(), bv.ap(), bp.ap(), out.ap())
    dbg = None
    if debug_taps:
        dbg = {
            "stats": nc.declare_dram_parameter("dbg_stats", [2, P, T], BF16, isOutput=True).ap(),
            "xn": nc.declare_dram_parameter("dbg_xn", [P, CCH, T], BF16, isOutput=True).ap(),
            "qk": nc.declare_dram_parameter("dbg_qk", [P, 8, T], BF16, isOutput=True).ap(),
            "v2": nc.declare_dram_parameter("dbg_v2", [P, 8, N_HEADS * P], BF16, isOutput=True).ap(),
            "exp": nc.declare_dram_parameter("dbg_exp", [2, P, T], FP8, isOutput=True).ap(),
            "rdb": nc.declare_dram_parameter("dbg_rdb", [2, HD, T], F32, isOutput=True).ap(),
            "h": nc.declare_dram_parameter("dbg_h", [P, CCH, T], BF16, isOutput=True).ap(),
        }

    with tile.TileContext(nc) as tc:
        import contextlib

        with contextlib.ExitStack() as ctx:
            pools = (
                ctx.enter_context(tc.tile_pool(name="const", bufs=1)),
                ctx.enter_context(tc.tile_pool(name="x", bufs=2)),
                ctx.enter_context(tc.tile_pool(name="x2", bufs=1)),
                ctx.enter_context(tc.tile_pool(name="xn", bufs=2)),
                ctx.enter_context(tc.tile_pool(name="stat", bufs=5)),
                ctx.enter_context(tc.tile_pool(name="qk", bufs=2)),
                ctx.enter_context(tc.tile_pool(name="h", bufs=2)),
                ctx.enter_context(tc.tile_pool(name="exp", bufs=4)),
                ctx.enter_context(tc.tile_pool(name="rds", bufs=2)),
                ctx.enter_context(tc.tile_pool(name="rdb", bufs=2)),
                ctx.enter_context(tc.tile_pool(name="out", bufs=2)),
                ctx.enter_context(tc.tile_pool(name="ps", bufs=2, space="PSUM")),
                ctx.enter_context(tc.tile_pool(name="acc", bufs=2, space="PSUM")),
                ctx.enter_context(tc.tile_pool(name="drd", bufs=4, space="DRAM")),
            )
            _emit(tc, nc, pools, aps, dbg)
    nc.compile()
    return nc


def _host_prep(w_qkv, b_qkv, w_proj, b_proj):
    rows = np.arange(3 * C).reshape(N_HEADS, 3, HD)
    qk_order = []
    for pc in range(4):
        qk_order += list(rows[2 * pc, 0]) + list(rows[2 * pc + 1, 0])
        qk_order += list(rows[2 * pc, 1]) + list(rows[2 * pc + 1, 1])
    qk_order = np.array(qk_order)
    v_order = rows[:, 2, :].reshape(-1)
    prep = {
        "w_qkT": np.ascontiguousarray(w_qkv[qk_order].T).astype(ml_dtypes.bfloat16),
        "w_vT": np.ascontiguousarray(w_qkv[v_order].T).astype(ml_dtypes.bfloat16),
        "w_projT": np.ascontiguousarray(w_proj.T).astype(ml_dtypes.bfloat16),
        "b_qk": np.ascontiguousarray(b_qkv[qk_order]).astype(np.float32),
        "b_v": np.ascontiguousarray(b_qkv[v_order]).astype(np.float32),
        "b_proj": np.ascontiguousarray(b_proj).astype(np.float32),
    }
    return prep


def _make_in_maps(x, w_qkv, b_qkv, w_proj, b_proj):
    prep = _host_prep(
        np.asarray(w_qkv, np.float32), np.asarray(b_qkv, np.float32),
        np.asarray(w_proj, np.float32), np.asarray(b_proj, np.float32),
    )
    xf = np.asarray(x, np.float32).reshape(B, C, T).astype(ml_dtypes.bfloat16)
    in_maps = []
    for core in range(N_CORES):
        m = dict(prep)
        m["x"] = np.ascontiguousarray(xf[core * B_LOC : (core + 1) * B_LOC])
        in_maps.append(m)
    return in_maps


_NC = None


def kernel(x, emb, w_qkv, b_qkv, w_proj, b_proj):
    global _NC
    x = np.asarray(x, dtype=np.float32)
    b, c, hh, ww = x.shape
    assert (b, c, hh * ww) == (B, C, T)
    if _NC is None:
        _NC = build_nc()
    in_maps = _make_in_maps(x, w_qkv, b_qkv, w_proj, b_proj)
    res = run_bass_kernel_spmd(_NC, in_maps, core_ids=list(range(N_CORES)), trace=False)
    out = np.concatenate([res.results[i]["out"] for i in range(N_CORES)], axis=0)
    return out.reshape(B, C, hh, ww).astype(np.float32)
